# revision 1
# baseline (speedup 1.0000x reference)
"""DisentangledSeqEncoder Trainium2 kernel.

Strategy (pure data-parallel over batch, 8 NeuronCores):
  - Each core gets B/8 = 64 batches; all small params replicated.
  - Token permutation: token t = 8*p + i lives at SBUF partition p,
    "chunk" index i in 0..7.  This makes the 2 MB per-group z DMA read
    2 KB-contiguous per partition (line-rate) and is transparent to the
    math (softmax over T and sum over T are permutation invariant).
  - LayerNorms are folded algebraically:
      scores_k[t,k] = inv_t * (z@G)[t,k] - inv_t*m_t*cg_k (+cb_k)
        with G = (pn^T * gamma0)/8, cg = colsum(G), pn = LN(prototypes)
      scores_t[t]  = inv2_t * ((z+a)@h - m2_t*sh) + const  (const dropped:
        softmax-invariant), h = gamma2*(q + W@q)/8, sh = sum(h)
      p_i normalization deferred: A = sum_t p_k|t * u_t * z_t accumulated
      in PSUM, divided by S = sum_t u_t at the end.
  - Per (group of 8 batches, chunk i): PE transposes z (pack-2), does the
    scores matmul (block-diag rhs with G and h columns), the (z+a)@h
    column via an alphas^T matmul, the u-sum, and an 8-batch-packed
    f32r aggregation matmul; DVE does bn_stats/softmax glue; ACT does
    transposed-z eviction, ln/exp (rsqrt via exp(-0.5*ln(x))); GPSIMD
    does broadcast adds/mults.
"""

import numpy as np

EPS = 1e-6
B_FULL, T, D, K = 512, 1024, 64, 16
NCORES = 8
B_CORE = B_FULL // NCORES          # 64
NG = 8                             # batch groups per core
NB = 8                             # batches per group
NI = 8                             # chunks (inner token index)
P = 128                            # partitions

_CACHE = {}


def _setup_act_tables():
    """Reorder act_func_sets so natural_log_exp_and_others is first: every
    Exp/Ln/Square/Copy activation then resolves to one table set and the
    per-chunk ACT_TABLE_LOAD thrash (~2.7us each) disappears.  Patches both
    the bass-side selection (get_activation_tables) and the walrus-side
    table dir (BASS_ACT_ROOT_JSON_PATH)."""
    import os
    import json
    import functools
    import concourse.hw_specs as hw_specs
    import concourse.bacc as bacc

    if getattr(_setup_act_tables, "_done", False):
        return
    from neuronxcc.driver.Job import Job
    from neuronxcc.driver.jobs.support.FindActInfo import findActInfoFile

    src = findActInfoFile(Job.getPackageDir(), "gen3")
    srcdir = os.path.dirname(src)
    info = json.load(open(src))
    sets = info["act_func_sets"]
    sets.sort(key=lambda e: 0 if e["name"] == "natural_log_exp_and_others" else 1)
    dst = "/tmp/act_reordered"
    os.makedirs(dst, exist_ok=True)
    tmp = os.path.join(dst, f"act_info.{os.getpid()}.tmp")
    json.dump(info, open(tmp, "w"))
    os.replace(tmp, os.path.join(dst, "act_info.json"))
    for f in os.listdir(srcdir):
        if f.endswith(".bin") or f.endswith(".json"):
            l = os.path.join(dst, f)
            if f != "act_info.json" and not os.path.exists(l):
                try:
                    os.symlink(os.path.join(srcdir, f), l)
                except FileExistsError:
                    pass
    os.environ["BASS_ACT_ROOT_JSON_PATH"] = os.path.join(dst, "act_info.json")

    orig = hw_specs.get_activation_tables

    @functools.cache
    def patched(arch):
        d = dict(orig(arch))
        items = list(d.items())
        items.sort(key=lambda kv: 0 if kv[0] == "natural_log_exp_and_others"
                   else 1)
        return dict(items)

    hw_specs.get_activation_tables = patched
    bacc.get_activation_tables = patched
    _setup_act_tables._done = True


def _emit(nc, z_d, proto_d, alphas_d, bbias_d, w_d, wb_d, gam_d, bet_d,
          bseq_d, ident_d, p2_d, rep16_d, ones_d, onesr_d, out_d, flags):
    import concourse.tile as tile
    import concourse.bass as bass
    from concourse import mybir

    f32 = mybir.dt.float32
    f32r = mybir.dt.float32r
    OP = mybir.AluOpType
    AF = mybir.ActivationFunctionType
    AX = mybir.AxisListType

    use_beta0 = flags["use_beta0"]

    def bcast_ap(src, p):
        # DRAM AP replicated p times along a new leading dim
        return bass.AP(tensor=src.tensor, offset=src.offset,
                       ap=[[0, p]] + [list(x) for x in src.ap])

    zf = z_d[:, :, :]
    alf = alphas_d[:, :]
    bsf = bseq_d[:, :]

    with tile.TileContext(nc) as tc:
        with (
            tc.tile_pool(name="singles", bufs=1) as singles,
            tc.tile_pool(name="zpool", bufs=2) as zpool,
            tc.tile_pool(name="gsb", bufs=3) as gsb,
            tc.tile_pool(name="chk", bufs=4) as chk,
            tc.tile_pool(name="chk2", bufs=4) as chk2,
            tc.tile_pool(name="psA", bufs=1, space="PSUM") as psA,   # ztp
            tc.tile_pool(name="psB", bufs=4, space="PSUM") as psB,   # scores
            tc.tile_pool(name="psAgg", bufs=1, space="PSUM") as psAgg,
            tc.tile_pool(name="psS", bufs=1, space="PSUM") as psS,
            tc.tile_pool(name="psSm", bufs=1, space="PSUM") as psSm,  # small
        ):
            # ============== startup constants ==============
            ident = singles.tile([P, P], f32)
            nc.gpsimd.dma_start(out=ident, in_=ident_d[:, :])
            p2 = singles.tile([D, P], f32)
            nc.gpsimd.dma_start(out=p2, in_=p2_d[:, :])
            rep16 = singles.tile([NB, P], f32)
            nc.gpsimd.dma_start(out=rep16, in_=rep16_d[:, :])
            ones_c = singles.tile([P, 1], f32)
            nc.gpsimd.dma_start(out=ones_c, in_=ones_d[:, :])
            ones_r = singles.tile([1, P], f32)
            nc.gpsimd.dma_start(out=ones_r, in_=onesr_d[:, :])

            epsc = singles.tile([P, 1], f32)
            nc.vector.memset(epsc, EPS)

            # small param columns
            g0col = singles.tile([D, 1], f32)
            nc.gpsimd.dma_start(out=g0col, in_=gam_d[0, :].unsqueeze(1))
            g2col = singles.tile([D, 1], f32)
            nc.gpsimd.dma_start(out=g2col, in_=gam_d[2, :].unsqueeze(1))
            nc.scalar.mul(out=g2col, in_=g2col, mul=0.125)

            # broadcast rows
            g1b = singles.tile([K, D], f32)
            nc.gpsimd.dma_start(out=g1b, in_=bcast_ap(gam_d[1, :], K))
            b1b = singles.tile([K, D], f32)
            nc.gpsimd.dma_start(out=b1b, in_=bcast_ap(bet_d[1, :], K))
            g3b = singles.tile([NB, D], f32)
            nc.gpsimd.dma_start(out=g3b, in_=bcast_ap(gam_d[3, :], NB))
            b3b = singles.tile([NB, D], f32)
            nc.gpsimd.dma_start(out=b3b, in_=bcast_ap(bet_d[3, :], NB))
            g4b = singles.tile([P, D], f32)
            nc.gpsimd.dma_start(out=g4b, in_=bcast_ap(gam_d[4, :], P))
            b4b = singles.tile([P, D], f32)
            nc.gpsimd.dma_start(out=b4b, in_=bcast_ap(bet_d[4, :], P))

            # ab8 = broadcast(alphas[-1] + b_bias)
            al8 = singles.tile([NB, D], f32)
            nc.gpsimd.dma_start(out=al8, in_=bcast_ap(alphas_d[T - 1, :], NB))
            bb8 = singles.tile([NB, D], f32)
            nc.gpsimd.dma_start(out=bb8, in_=bcast_ap(bbias_d[:], NB))
            ab8 = singles.tile([NB, D], f32)
            nc.vector.tensor_add(out=ab8, in0=al8, in1=bb8)

            # beta_seq replicated 8x on partitions
            bsqrep = singles.tile([P, D], f32)
            nc.gpsimd.dma_start(
                out=bsqrep,
                in_=bass.AP(tensor=bsf.tensor, offset=bsf.offset,
                            ap=[[0, NG], [D, K], [1, D]]))

            # W^T
            w_s = singles.tile([D, D], f32)
            nc.gpsimd.dma_start(out=w_s, in_=w_d[:, :])
            wtp = psSm.tile([D, D], f32, tag="sm")
            nc.tensor.transpose(wtp, w_s, ident[0:D, 0:D])
            wt_s = singles.tile([D, D], f32)
            nc.scalar.copy(out=wt_s, in_=wtp)

            # prototypes -> pn = LN(proto)*g1 + b1
            proto_s = singles.tile([K, D], f32)
            nc.gpsimd.dma_start(out=proto_s, in_=proto_d[:, :])
            pst = singles.tile([K, 6], f32)
            nc.vector.bn_stats(out=pst, in_=proto_s)
            pmv = singles.tile([K, 2], f32)
            nc.vector.bn_aggr(out=pmv, in_=pst)
            plv = singles.tile([K, 1], f32)
            nc.scalar.activation(out=plv, in_=pmv[:, 1:2], func=AF.Ln,
                                 bias=epsc[0:K], scale=1.0)
            pinv = singles.tile([K, 1], f32)
            nc.scalar.activation(out=pinv, in_=plv, func=AF.Exp, scale=-0.5)
            pn = singles.tile([K, D], f32)
            nc.vector.tensor_scalar(out=pn, in0=proto_s, scalar1=pmv[:, 0:1],
                                    scalar2=pinv, op0=OP.subtract, op1=OP.mult)
            nc.vector.tensor_mul(out=pn, in0=pn, in1=g1b)
            nc.vector.tensor_add(out=pn, in0=pn, in1=b1b)

            # G = pn^T * g0 / 8  [D, K]
            pntp = psSm.tile([D, K], f32, tag="sm")
            nc.tensor.transpose(pntp, pn, ident[0:K, 0:K])
            g_s = singles.tile([D, K], f32)
            nc.vector.tensor_scalar(out=g_s, in0=pntp, scalar1=g0col,
                                    scalar2=0.125, op0=OP.mult, op1=OP.mult)
            # G2 = [G; G] on 128 partitions
            g2p = psSm.tile([P, K], f32, tag="sm")
            nc.tensor.matmul(g2p, lhsT=p2, rhs=g_s, start=True, stop=True)
            g2_s = singles.tile([P, K], f32)
            nc.scalar.copy(out=g2_s, in_=g2p)

            # cg row, cgb' = -0.5 * broadcast(colsum(G))
            cgp = psSm.tile([1, K], f32, tag="sm")
            nc.tensor.matmul(cgp, lhsT=ones_c[0:D, 0:1], rhs=g_s,
                             start=True, stop=True)
            cgr = singles.tile([1, K], f32)
            nc.scalar.copy(out=cgr, in_=cgp)
            cgbp = psSm.tile([P, K], f32, tag="sm")
            nc.tensor.matmul(cgbp, lhsT=ones_r, rhs=cgr, start=True, stop=True)
            cgb = singles.tile([P, K], f32)
            nc.scalar.mul(out=cgb, in_=cgbp, mul=-1.0 / 64.0)

            ecb = None
            if use_beta0:
                b0col = singles.tile([D, 1], f32)
                nc.gpsimd.dma_start(out=b0col, in_=bet_d[0, :].unsqueeze(1))
                bpn = singles.tile([D, K], f32)
                nc.vector.tensor_scalar(out=bpn, in0=pntp, scalar1=b0col,
                                        scalar2=0.125, op0=OP.mult, op1=OP.mult)
                cbp = psSm.tile([1, K], f32, tag="sm")
                nc.tensor.matmul(cbp, lhsT=ones_c[0:D, 0:1], rhs=bpn,
                                 start=True, stop=True)
                cbr = singles.tile([1, K], f32)
                nc.scalar.activation(out=cbr, in_=cbp, func=AF.Exp, scale=1.0)
                ecbp = psSm.tile([P, K], f32, tag="sm")
                nc.tensor.matmul(ecbp, lhsT=ones_r, rhs=cbr, start=True, stop=True)
                ecb = singles.tile([P, K], f32)
                nc.scalar.copy(out=ecb, in_=ecbp)

            # alphas in permuted layout + per-chunk transposes
            a_nat = singles.tile([P, NI, D], f32)
            nc.sync.dma_start(
                out=a_nat,
                in_=bass.AP(tensor=alf.tensor, offset=alf.offset,
                            ap=[[NI * D, P], [D, NI], [1, D]]))
            aT = singles.tile([D, NI, P], f32)
            for i in range(NI):
                atp = psSm.tile([D, P], f32, tag="sm", name=f"atp{i}")
                nc.tensor.transpose(atp, a_nat[:, i, :], ident)
                nc.scalar.copy(out=aT[:, i, :], in_=atp)
            ras = singles.tile([P, NI], f32)
            nc.vector.reduce_sum(out=ras, in_=a_nat, axis=AX.X)
            asq = singles.tile([P, NI, D], f32)
            nc.scalar.activation(out=asq, in_=a_nat, func=AF.Square)
            ras2 = singles.tile([P, NI], f32)
            nc.vector.reduce_sum(out=ras2, in_=asq, axis=AX.X)

            # scores rhs tiles R_j [128, 2, 17]: G blocks + per-group h col
            Rt = []
            for j in range(4):
                r = singles.tile([P, 2, 18], f32, name=f"Rt{j}")
                nc.vector.memset(r, 0.0)
                nc.vector.tensor_copy(out=r[0:D, 0, 0:K], in_=g2_s[0:D])
                nc.vector.tensor_copy(out=r[D:P, 1, 0:K], in_=g2_s[D:P])
                nc.vector.memset(r[0:D, 0:1, 17:18], 1.0)
                nc.vector.memset(r[D:P, 1:2, 17:18], 1.0)
                Rt.append(r)

            # ============== per-group loop ==============
            for g in range(NG):
                zg = zpool.tile([P, NI, NB, D], f32, name="zg")
                nc.sync.dma_start(
                    out=zg,
                    in_=bass.AP(tensor=zf.tensor,
                                offset=zf.offset + g * NB * T * D,
                                ap=[[NI * D, P], [D, NI], [T * D, NB], [1, D]]))

                # ---- q chain ----
                zl = gsb.tile([NB, D], f32, name="zl")
                nc.gpsimd.dma_start(
                    out=zl,
                    in_=bass.AP(tensor=zf.tensor,
                                offset=zf.offset + g * NB * T * D + (T - 1) * D,
                                ap=[[T * D, NB], [1, D]]))
                qin = gsb.tile([NB, D], f32, name="qin")
                nc.vector.tensor_add(out=qin, in0=zl, in1=ab8)
                qst = gsb.tile([NB, 6], f32, name="qst")
                nc.vector.bn_stats(out=qst, in_=qin)
                qmv = gsb.tile([NB, 2], f32, name="qmv")
                nc.vector.bn_aggr(out=qmv, in_=qst)
                qlv = gsb.tile([NB, 1], f32, name="qlv")
                nc.scalar.activation(out=qlv, in_=qmv[:, 1:2], func=AF.Ln,
                                     bias=epsc[0:NB], scale=1.0)
                qiv = gsb.tile([NB, 1], f32, name="qiv")
                nc.scalar.activation(out=qiv, in_=qlv, func=AF.Exp, scale=-0.5)
                q_t = gsb.tile([NB, D], f32, name="q_t")
                nc.vector.tensor_scalar(out=q_t, in0=qin, scalar1=qmv[:, 0:1],
                                        scalar2=qiv, op0=OP.subtract, op1=OP.mult)
                nc.vector.tensor_mul(out=q_t, in0=q_t, in1=g3b)
                nc.vector.tensor_add(out=q_t, in0=q_t, in1=b3b)
                qtp = psSm.tile([D, NB], f32, tag="sm", name="qtp")
                nc.tensor.transpose(qtp, q_t, ident[0:NB, 0:NB])
                qts = gsb.tile([D, NB], f32, name="qts")
                nc.scalar.copy(out=qts, in_=qtp)
                qwp = psSm.tile([D, NB], f32, tag="sm", name="qwp")
                nc.tensor.matmul(qwp, lhsT=wt_s, rhs=qts, start=True, stop=True)
                qeT = gsb.tile([D, NB], f32, name="qeT")
                nc.vector.tensor_add(out=qeT, in0=qts, in1=qwp)
                hTs = gsb.tile([D, NB], f32, name="hTs")
                nc.vector.tensor_scalar_mul(out=hTs, in0=qeT, scalar1=g2col)
                h2p = psSm.tile([P, NB], f32, tag="sm", name="h2p")
                nc.tensor.matmul(h2p, lhsT=p2, rhs=hTs, start=True, stop=True)
                hT2 = gsb.tile([P, NB], f32, name="hT2")
                nc.scalar.copy(out=hT2, in_=h2p)
                shp = psSm.tile([1, NB], f32, tag="sm", name="shp")
                nc.tensor.matmul(shp, lhsT=ones_c[0:D, 0:1], rhs=hTs,
                                 start=True, stop=True)
                shr = gsb.tile([1, NB], f32, name="shr")
                nc.scalar.copy(out=shr, in_=shp)
                shBp = psSm.tile([P, NB], f32, tag="sm", name="shBp")
                nc.tensor.matmul(shBp, lhsT=ones_r, rhs=shr, start=True, stop=True)
                shB = gsb.tile([P, NB], f32, name="shB")
                nc.scalar.mul(out=shB, in_=shBp, mul=1.0 / 64.0)

                # h columns into R tiles
                for j in range(4):
                    nc.vector.tensor_copy(out=Rt[j][0:D, 0, 16:17],
                                          in_=hT2[0:D, 2 * j:2 * j + 1])
                    nc.vector.tensor_copy(out=Rt[j][D:P, 1, 16:17],
                                          in_=hT2[D:P, 2 * j + 1:2 * j + 2])

                aggp = psAgg.tile([D, NB * K], f32, name="aggp")
                sp = psS.tile([NB, 1], f32, name="sp")

                # ---- chunk loop (processed in pairs) ----
                for ci in range(NI // 2):
                    scps = []
                    rz2p = chk2.tile([P, 2, NB], f32, name="rz2p")
                    rcxp = chk2.tile([P, 2, NB], f32, name="rcxp")
                    rzsp = chk2.tile([P, 2, NB], f32, name="rzsp")
                    for cc in range(2):
                        i = 2 * ci + cc
                        zc = zg[:, i, :, :]
                        ztp = psA.tile([P, 4 * P], f32, name="ztp")
                        for j in range(4):
                            nc.tensor.transpose(
                                ztp[:, j * P:(j + 1) * P],
                                zg[:, i, 2 * j:2 * j + 2, :].rearrange(
                                    "p b d -> p (b d)"), ident)
                        zs8 = chk.tile([P, 4 * P], f32, name="zs8")
                        nc.scalar.copy(out=zs8, in_=ztp)

                        scp = psB.tile([P, NB, 18], f32, name="scp")
                        nc.tensor.matmul(scp[:, :, 16], lhsT=aT[:, i, :],
                                         rhs=hT2[0:D, :], start=True,
                                         stop=False, skip_group_check=True)
                        for j in range(4):
                            nc.tensor.matmul(
                                scp[:, 2 * j:2 * j + 2, :].rearrange(
                                    "p a b -> p (a b)"),
                                lhsT=zs8[:, j * P:(j + 1) * P],
                                rhs=Rt[j].rearrange("p a b -> p (a b)"),
                                start=False, stop=(j == 3),
                                skip_group_check=True)
                        scps.append(scp)

                        azp = chk.tile([P, NB, D], f32, name="azp")
                        nc.gpsimd.tensor_mul(
                            out=azp, in0=zc,
                            in1=a_nat[:, i, :].unsqueeze(1).broadcast_to(
                                (P, NB, D)))
                        zsq = chk.tile([P, NB, D], f32, name="zsq")
                        nc.scalar.activation(out=zsq, in_=zc, func=AF.Square)
                        nc.vector.reduce_sum(out=rz2p[:, cc, :], in_=zsq,
                                             axis=AX.X)
                        nc.vector.reduce_sum(out=rcxp[:, cc, :], in_=azp,
                                             axis=AX.X)
                        nc.scalar.copy(out=rzsp[:, cc, :], in_=scp[:, :, 17])

                    # pair-level stat combines on [P, 2, NB]
                    vvp = chk2.tile([P, 2, 2, NB], f32, name="vvp")
                    t0p = chk2.tile([P, 2, NB], f32, name="t0p")
                    nc.gpsimd.tensor_mul(out=t0p, in0=rzsp, in1=rzsp)
                    nc.vector.scalar_tensor_tensor(
                        out=vvp[:, 0], in0=t0p, scalar=-1.0 / 64.0, in1=rz2p,
                        op0=OP.mult, op1=OP.add)
                    mazp = chk2.tile([P, 2, NB], f32, name="mazp")
                    nc.gpsimd.tensor_add(
                        out=mazp, in0=rzsp,
                        in1=ras[:, 2 * ci:2 * ci + 2].unsqueeze(2)
                        .broadcast_to((P, 2, NB)))
                    t5p = chk2.tile([P, 2, NB], f32, name="t5p")
                    nc.vector.scalar_tensor_tensor(
                        out=t5p, in0=rcxp, scalar=2.0, in1=rz2p,
                        op0=OP.mult, op1=OP.add)
                    nc.gpsimd.tensor_add(
                        out=t5p, in0=t5p,
                        in1=ras2[:, 2 * ci:2 * ci + 2].unsqueeze(2)
                        .broadcast_to((P, 2, NB)))
                    t6p = chk2.tile([P, 2, NB], f32, name="t6p")
                    nc.gpsimd.tensor_mul(out=t6p, in0=mazp, in1=mazp)
                    nc.vector.scalar_tensor_tensor(
                        out=vvp[:, 1], in0=t6p, scalar=-1.0 / 64.0, in1=t5p,
                        op0=OP.mult, op1=OP.add)
                    lnvp = chk2.tile([P, 2, 2, NB], f32, name="lnvp")
                    nc.scalar.activation(out=lnvp, in_=vvp, func=AF.Ln,
                                         bias=epsc, scale=1.0 / 64.0)
                    ivp = chk2.tile([P, 2, 2, NB], f32, name="ivp")
                    nc.scalar.activation(out=ivp, in_=lnvp, func=AF.Exp,
                                         scale=-0.5)
                    t2p = chk2.tile([P, 2, NB], f32, name="t2p")
                    nc.gpsimd.tensor_mul(
                        out=t2p, in0=mazp,
                        in1=shB.unsqueeze(1).broadcast_to((P, 2, NB)))

                    for cc in range(2):
                        i = 2 * ci + cc
                        zc = zg[:, i, :, :]
                        scp = scps[cc]
                        inv = ivp[:, 0, cc, :]
                        inv2 = ivp[:, 1, cc, :]
                        stile = chk.tile([P, NB, 17], f32, name="stile")
                        mcg = chk.tile([P, NB, K], f32, name="mcg")
                        nc.gpsimd.tensor_tensor(
                            out=mcg,
                            in0=cgb.unsqueeze(1).broadcast_to((P, NB, K)),
                            in1=rzsp[:, cc, :].unsqueeze(2).broadcast_to(
                                (P, NB, K)),
                            op=OP.mult)
                        nc.vector.tensor_add(out=stile[:, :, 0:K],
                                             in0=scp[:, :, 0:K], in1=mcg)
                        nc.vector.tensor_tensor(
                            out=stile[:, :, 0:K], in0=stile[:, :, 0:K],
                            in1=inv.unsqueeze(2).broadcast_to((P, NB, K)),
                            op=OP.mult)
                        azh = chk2.tile([P, NB], f32, name="azh")
                        nc.vector.tensor_sub(out=azh, in0=scp[:, :, 16],
                                             in1=t2p[:, cc, :])
                        nc.vector.tensor_mul(out=stile[:, :, 16], in0=azh,
                                             in1=inv2)

                        etile = chk.tile([P, NB, 17], f32, name="etile")
                        nc.scalar.activation(out=etile, in_=stile, func=AF.Exp)
                        ev = etile[:, :, 0:K]
                        if use_beta0:
                            nc.vector.tensor_tensor(
                                out=ev, in0=ev,
                                in1=ecb.unsqueeze(1).broadcast_to((P, NB, K)),
                                op=OP.mult)
                        sk = chk2.tile([P, NB], f32, name="sk")
                        nc.vector.reduce_sum(out=sk, in_=ev, axis=AX.X)
                        rk = chk2.tile([P, NB], f32, name="rk")
                        nc.vector.reciprocal(out=rk, in_=sk)
                        nc.vector.tensor_mul(out=rk, in0=rk,
                                             in1=etile[:, :, 16])
                        wt = chk.tile([P, NB, K], f32, name="wt")
                        nc.vector.tensor_tensor(
                            out=wt, in0=ev,
                            in1=rk.unsqueeze(2).broadcast_to((P, NB, K)),
                            op=OP.mult)

                        nc.tensor.matmul(sp, lhsT=etile[:, :, 16], rhs=ones_c,
                                         start=(i == 0), stop=(i == NI - 1))
                        for b in range(NB):
                            nc.tensor.matmul(
                                aggp[:, K * b:K * (b + 1)],
                                lhsT=zc[:, b, :], rhs=wt[:, b, :],
                                start=(i == 0 and b == 0),
                                stop=(i == NI - 1 and b == NB - 1),
                                skip_group_check=True)

                # ---- group tail: normalize + final LN + out ----
                srec = gsb.tile([NB, 1], f32, name="srec")
                nc.vector.reciprocal(out=srec, in_=sp)
                srp = psSm.tile([P, 1], f32, tag="sm", name="srp")
                nc.tensor.matmul(srp, lhsT=rep16, rhs=srec, start=True, stop=True)
                srr = gsb.tile([P, 1], f32, name="srr")
                nc.scalar.copy(out=srr, in_=srp)

                ats = gsb.tile([D, NB * K], f32, name="ats")
                nc.scalar.copy(out=ats, in_=aggp)
                atp2 = psSm.tile([P, D], f32, tag="sm", name="atp2")
                nc.tensor.transpose(atp2, ats, ident[0:D, 0:D])
                a8 = gsb.tile([P, D], f32, name="a8")
                nc.vector.scalar_tensor_tensor(
                    out=a8, in0=atp2, scalar=srr, in1=bsqrep,
                    op0=OP.mult, op1=OP.add)
                fst = gsb.tile([P, 6], f32, name="fst")
                nc.vector.bn_stats(out=fst, in_=a8)
                fmv = gsb.tile([P, 2], f32, name="fmv")
                nc.vector.bn_aggr(out=fmv, in_=fst)
                flv = gsb.tile([P, 1], f32, name="flv")
                nc.scalar.activation(out=flv, in_=fmv[:, 1:2], func=AF.Ln,
                                     bias=epsc, scale=1.0)
                fiv = gsb.tile([P, 1], f32, name="fiv")
                nc.scalar.activation(out=fiv, in_=flv, func=AF.Exp, scale=-0.5)
                obuf = gsb.tile([P, D], f32, name="obuf")
                nc.vector.tensor_scalar(out=obuf, in0=a8, scalar1=fmv[:, 0:1],
                                        scalar2=fiv, op0=OP.subtract, op1=OP.mult)
                nc.vector.tensor_mul(out=obuf, in0=obuf, in1=g4b)
                nc.vector.tensor_add(out=obuf, in0=obuf, in1=b4b)
                nc.sync.dma_start(
                    out=out_d[g * NB:(g + 1) * NB].flatten_outer_dims(),
                    in_=obuf)

    return nc


def _build(flags):
    import concourse.bacc as bacc
    from concourse import mybir

    _setup_act_tables()
    f32 = mybir.dt.float32
    nc = bacc.Bacc("TRN2", target_bir_lowering=False, debug=False,
                   num_devices=NCORES)
    dp = nc.declare_dram_parameter
    hs = [
        dp("z", [B_CORE, T, D], f32, isOutput=False),
        dp("prototypes", [K, D], f32, isOutput=False),
        dp("alphas", [T, D], f32, isOutput=False),
        dp("b_bias", [D], f32, isOutput=False),
        dp("W", [D, D], f32, isOutput=False),
        dp("Wb", [D], f32, isOutput=False),
        dp("ln_gamma", [5, D], f32, isOutput=False),
        dp("ln_beta", [5, D], f32, isOutput=False),
        dp("beta_seq", [K, D], f32, isOutput=False),
        dp("c_ident", [P, P], f32, isOutput=False),
        dp("c_p2", [D, P], f32, isOutput=False),
        dp("c_rep16", [NB, P], f32, isOutput=False),
        dp("c_ones", [P, 1], f32, isOutput=False),
        dp("c_onesr", [1, P], f32, isOutput=False),
    ]
    out_d = dp("out", [B_CORE, K, D], f32, isOutput=True)
    _emit(nc, *hs, out_d, flags)
    nc.finalize()
    return nc


def _consts():
    ident = np.eye(P, dtype=np.float32)
    p2 = np.concatenate([np.eye(D, dtype=np.float32),
                         np.eye(D, dtype=np.float32)], axis=1)  # [64, 128]
    rep16 = np.zeros((NB, P), dtype=np.float32)
    for j in range(NB):
        rep16[j, j * K:(j + 1) * K] = 1.0
    ones_c = np.ones((P, 1), dtype=np.float32)
    ones_r = np.ones((1, P), dtype=np.float32)
    return ident, p2, rep16, ones_c, ones_r


def kernel(**inputs):
    from concourse.bass_utils import run_bass_kernel_spmd

    z = np.ascontiguousarray(inputs["z"], dtype=np.float32)
    flags = {
        "use_beta0": bool(np.abs(np.asarray(inputs["ln_beta"])[0]).max() > 0),
    }
    key = tuple(sorted(flags.items()))
    if key not in _CACHE:
        _CACHE[key] = _build(flags)
    nc = _CACHE[key]

    ident, p2, rep16, ones_c, ones_r = _consts()
    common = {
        "prototypes": np.ascontiguousarray(inputs["prototypes"], np.float32),
        "alphas": np.ascontiguousarray(inputs["alphas"], np.float32),
        "b_bias": np.ascontiguousarray(inputs["b_bias"], np.float32),
        "W": np.ascontiguousarray(inputs["W"], np.float32),
        "Wb": np.ascontiguousarray(inputs["Wb"], np.float32),
        "ln_gamma": np.ascontiguousarray(inputs["ln_gamma"], np.float32),
        "ln_beta": np.ascontiguousarray(inputs["ln_beta"], np.float32),
        "beta_seq": np.ascontiguousarray(inputs["beta_seq"], np.float32),
        "c_ident": ident, "c_p2": p2, "c_rep16": rep16,
        "c_ones": ones_c, "c_onesr": ones_r,
    }
    in_maps = []
    for c in range(NCORES):
        m = dict(common)
        m["z"] = np.ascontiguousarray(z[c * B_CORE:(c + 1) * B_CORE])
        in_maps.append(m)
    res = run_bass_kernel_spmd(nc, in_maps, core_ids=list(range(NCORES)))
    out = np.concatenate([r["out"] for r in res.results], axis=0)
    return out



# revision 8
# speedup vs baseline: 2.9511x; 2.9511x over previous
"""DisentangledSeqEncoder Trainium2 kernel, v2.

Strategy (pure data-parallel over batch, 8 NeuronCores):
  - Host sends z in TWO bf16 layouts (natural + transposed), both with
    >= 8KB-contiguous rows so every big DMA runs at full bus rate.
    Token t = 8*tau + i lives at partition tau, chunk i in 0..7.
  - All per-(token,batch) reductions are PE matmul columns against the
    transposed z:
      * scores_k  = z @ Gc          (Gc column-centered: kills the m*cg term)
      * u-col     = (z+a) @ hc      (hc = C(I+W)q with C = I - J/64: the
                                     centering kills the m*sh term exactly)
      * moments   = [Sz/64, S(z+a)/64, Sz^2/64, S((z+a)^2)/64] via matmuls
        on z, z*z (DVE/ACT elementwise), and a*z (DVE), plus a constant
        matmul adding alpha-only terms.
  - Softmax: var -> Ln/Exp rsqrt on ACT; scale on Pool (PSUM-read);
    exp on ACT; k-sum on Pool; u/sk and ev*(u/sk) on DVE (bf16 2x mode).
  - Aggregation: per (chunk,batch) matmul lhsT=z_nat rhs=wt -> [d,(b,k)]
    PSUM accumulated over all 1024 tokens; group tail does the final LN.
  - The whole q->h chain runs ONCE for all 8 groups at startup.
  - gamma/beta LN params are folded on the host into Gc/g2col (exact);
    runtime flags add extra ops only when beta0 / gamma3,beta3 /
    gamma4,beta4 are nontrivial (they are ones/zeros here).
"""

import numpy as np

EPS = 1e-6
B_FULL, T, D, K = 512, 1024, 64, 16
NCORES = 8
B_CORE = B_FULL // NCORES          # 64
NG = 8                             # batch groups per core
NB = 8                             # batches per group
NI = 8                             # chunks (inner token index)
P = 128                            # partitions

_CACHE = {}


def _setup_act_tables():
    """Reorder act_func_sets so natural_log_exp_and_others is first (avoids
    per-chunk ACT_TABLE_LOAD thrash on real hw)."""
    import os
    import json
    import functools
    import concourse.hw_specs as hw_specs
    import concourse.bacc as bacc

    if getattr(_setup_act_tables, "_done", False):
        return
    from neuronxcc.driver.Job import Job
    from neuronxcc.driver.jobs.support.FindActInfo import findActInfoFile

    src = findActInfoFile(Job.getPackageDir(), "gen3")
    srcdir = os.path.dirname(src)
    info = json.load(open(src))
    sets = info["act_func_sets"]
    sets.sort(key=lambda e: 0 if e["name"] == "natural_log_exp_and_others" else 1)
    dst = "/tmp/act_reordered"
    os.makedirs(dst, exist_ok=True)
    tmp = os.path.join(dst, f"act_info.{os.getpid()}.tmp")
    json.dump(info, open(tmp, "w"))
    os.replace(tmp, os.path.join(dst, "act_info.json"))
    for f in os.listdir(srcdir):
        if f.endswith(".bin") or f.endswith(".json"):
            l = os.path.join(dst, f)
            if f != "act_info.json" and not os.path.exists(l):
                try:
                    os.symlink(os.path.join(srcdir, f), l)
                except FileExistsError:
                    pass
    os.environ["BASS_ACT_ROOT_JSON_PATH"] = os.path.join(dst, "act_info.json")

    orig = hw_specs.get_activation_tables

    @functools.cache
    def patched(arch):
        d = dict(orig(arch))
        items = list(d.items())
        items.sort(key=lambda kv: 0 if kv[0] == "natural_log_exp_and_others"
                   else 1)
        return dict(items)

    hw_specs.get_activation_tables = patched
    bacc.get_activation_tables = patched
    _setup_act_tables._done = True


def _emit(nc, zg_d, zT_d, pbf_d, pf3_d, out_d, flags, bfc, f3c):
    import concourse.tile as tile
    import concourse.bass as bass
    from concourse import mybir

    f32 = mybir.dt.float32
    bf16 = mybir.dt.bfloat16
    OP = mybir.AluOpType
    AF = mybir.ActivationFunctionType
    AX = mybir.AxisListType

    NBF = bfc["_total"]
    NF3 = f3c["_total"]

    with tile.TileContext(nc) as tc:
        with (
            tc.tile_pool(name="singles", bufs=1) as singles,
            tc.tile_pool(name="zn", bufs=2) as znp,
            tc.tile_pool(name="zt", bufs=2) as ztp_pool,
            tc.tile_pool(name="prod", bufs=3) as prod,
            tc.tile_pool(name="sfm", bufs=3) as sfm,
            tc.tile_pool(name="gsb", bufs=2) as gsb,
            tc.tile_pool(name="psS", bufs=2, space="PSUM") as psS,
            tc.tile_pool(name="psC", bufs=2, space="PSUM") as psC,   # scores
            tc.tile_pool(name="psAgg", bufs=2, space="PSUM") as psAgg,
            tc.tile_pool(name="psSm", bufs=2, space="PSUM") as psSm,
        ):
            # ================= startup =================
            pbf = singles.tile([P, NBF], bf16)
            nc.sync.dma_start(out=pbf, in_=pbf_d[:, :])
            pf3 = singles.tile([P, NF3], f32)
            nc.sync.dma_start(out=pf3, in_=pf3_d[:, :])

            def bfv(name, rows=P):
                off, ncol = bfc[name]
                return pbf[0:rows, off:off + ncol]

            def f3v(name, rows=P):
                off, ncol = f3c[name]
                return pf3[0:rows, off:off + ncol]

            aT2 = bfv("aT2").rearrange("p (i t) -> p i t", i=NI)
            rq = bfv("rq")
            raz = bfv("raz")
            RG = bfv("RG")
            constM = bfv("constM", rows=8).rearrange("p (q t) -> p q t", q=2)
            selC = bfv("selC", rows=8)
            WIC = bfv("WIC", rows=D)
            idbf = bfv("idbf", rows=D)
            ones_bf = bfv("ones")

            zlast = f3v("zlast", rows=D)
            ab8rep = f3v("ab8rep", rows=D)
            g2col = f3v("g2col", rows=D)
            ident = f3v("ident", rows=D)
            rep16 = f3v("rep16", rows=NB)
            bsqrep = f3v("bsqrep")

            epsc = singles.tile([P, 1], f32)
            nc.vector.memset(epsc, EPS)

            # ---- q -> hc chain, once for all 64 (g,b) ----
            qin = singles.tile([D, D], f32)
            nc.vector.tensor_add(out=qin, in0=zlast, in1=ab8rep)
            qst = singles.tile([D, 6], f32)
            nc.vector.bn_stats(out=qst, in_=qin)
            qmv = singles.tile([D, 2], f32)
            nc.vector.bn_aggr(out=qmv, in_=qst)
            qlv = singles.tile([D, 1], f32)
            nc.scalar.activation(out=qlv, in_=qmv[:, 1:2], func=AF.Ln,
                                 bias=epsc[0:D], scale=1.0)
            qiv = singles.tile([D, 1], f32)
            nc.scalar.activation(out=qiv, in_=qlv, func=AF.Exp, scale=-0.5)
            q_t = singles.tile([D, D], f32)
            nc.vector.tensor_scalar(out=q_t, in0=qin, scalar1=qmv[:, 0:1],
                                    scalar2=qiv, op0=OP.subtract, op1=OP.mult)
            if flags["use_g3b3"]:
                nc.vector.tensor_mul(out=q_t, in0=q_t, in1=f3v("g3rep", rows=D))
                nc.vector.tensor_add(out=q_t, in0=q_t, in1=f3v("b3rep", rows=D))
            qtp = psSm.tile([D, D], f32, tag="sm")
            nc.tensor.transpose(qtp, q_t, ident)
            qts = singles.tile([D, D], bf16)
            nc.scalar.copy(out=qts, in_=qtp)
            h1p = psSm.tile([D, D], f32, tag="sm")
            nc.tensor.matmul(h1p, lhsT=WIC, rhs=qts, start=True, stop=True)
            hT8 = singles.tile([D, D], bf16)
            nc.vector.tensor_scalar_mul(out=hT8, in0=h1p, scalar1=g2col)
            # block-diag h columns for the u-col matmuls: [(b2,d), g, j, b2']
            hcp = psSm.tile([P, NG, 4, 2], f32, tag="sm")
            nc.tensor.matmul(
                hcp[0:D, :, :, 0].rearrange("p a b -> p (a b)"), lhsT=idbf,
                rhs=hT8[:, 0::2], start=True, stop=True,
                skip_group_check=True)
            nc.tensor.matmul(
                hcp[D:P, :, :, 1].rearrange("p a b -> p (a b)"),
                lhsT=idbf, rhs=hT8[:, 1::2], start=True, stop=True,
                skip_group_check=True)
            nc.vector.memset(hcp[0:D, :, :, 1], 0.0)
            nc.vector.memset(hcp[D:P, :, :, 0], 0.0)
            hcall = singles.tile([P, NG, 4, 2], bf16)
            nc.scalar.copy(out=hcall, in_=hcp)

            # ================= group loop =================
            for g in range(NG):
                zTf = ztp_pool.tile([P, NI * 4 * P], bf16, name="zTf")
                nc.sync.dma_start(out=zTf, in_=zT_d[g, :, :])
                zT = zTf.rearrange("p (j i t) -> p j i t", j=4, i=NI)
                zgf = znp.tile([P, NB * NI * D], bf16, name="zgf")
                nc.sync.dma_start(out=zgf, in_=zg_d[g, :, :])
                zg = zgf.rearrange("p (b i d) -> p b i d", b=NB, i=NI)

                hT8g = hT8[:, g * NB:(g + 1) * NB]
                aggc = psAgg.tile([P, NB * K + 1], f32, name="aggc")
                aggp = aggc[0:D, 0:NB * K].rearrange("p (b k) -> p b k", b=NB)
                spp = aggc[0:NB, NB * K:NB * K + 1]

                for qd in range(2):
                    S2 = psS.tile([P, 5, 4, NB], f32, name="S2")
                    S = S2[:, 0:4, :, :]
                    U = S2[:, 4, :, :]
                    scp = psC.tile([P, 4, K, NB], f32, name="scp")

                    zzp = prod.tile([P, 4, 2, 2, P], bf16, name="zzp", tag="zzp")
                    azp = prod.tile([P, 4, 2, 2, P], bf16, name="azp", tag="azp")
                    for cc in range(2):
                        i0 = 4 * qd + 2 * cc
                        zsl = zT[:, :, i0:i0 + 2, :]
                        asl = aT2[:, i0:i0 + 2, :].unsqueeze(1) \
                            .broadcast_to((P, 4, 2, P))
                        # z*z : half on ACT (Square), half on DVE
                        nc.scalar.activation(out=zzp[:, 0:2, cc],
                                             in_=zT[:, 0:2, i0:i0 + 2, :],
                                             func=AF.Square)
                        nc.vector.tensor_mul(out=zzp[:, 2:4, cc],
                                             in0=zT[:, 2:4, i0:i0 + 2, :],
                                             in1=zT[:, 2:4, i0:i0 + 2, :])
                        # a*z on DVE (bf16 2x)
                        nc.vector.tensor_tensor(out=azp[:, :, cc], in0=zsl,
                                                in1=asl, op=OP.mult)

                        for c01 in range(2):
                            i = i0 + c01
                            ch = 2 * cc + c01
                            for j in range(4):
                                # moments: q0=Sz/64, q1=S(z+a)/64 (partial),
                                # q2=Sz2/64, q3=S((z+a)^2)/64 (partial)
                                nc.tensor.matmul(
                                    S[:, 0:2, ch, 2 * j:2 * j + 2],
                                    lhsT=zT[:, j, i, :], rhs=rq,
                                    start=True, stop=False,
                                    skip_group_check=True)
                                nc.tensor.matmul(
                                    S[:, 2:4, ch, 2 * j:2 * j + 2],
                                    lhsT=zzp[:, j, cc, c01, :], rhs=rq,
                                    start=True, stop=False,
                                    skip_group_check=True)
                                nc.tensor.matmul(
                                    S[:, 3, ch, 2 * j:2 * j + 2],
                                    lhsT=azp[:, j, cc, c01, :], rhs=raz,
                                    start=False, stop=False,
                                    skip_group_check=True)
                                # scores: z @ Gc
                                nc.tensor.matmul(
                                    scp[:, ch, :, 2 * j:2 * j + 2],
                                    lhsT=zT[:, j, i, :], rhs=RG,
                                    start=True, stop=True,
                                    skip_group_check=True)
                                # u-col: z @ hc (block-diag h cols)
                                nc.tensor.matmul(
                                    U[:, ch, 2 * j:2 * j + 2],
                                    lhsT=zT[:, j, i, :],
                                    rhs=hcall[:, g, j, :],
                                    start=True, stop=False,
                                    skip_group_check=True)
                            # u-col: + a @ hc
                            nc.tensor.matmul(
                                U[:, ch, :], lhsT=aT2[0:D, i, :], rhs=hT8g,
                                start=False, stop=True, skip_group_check=True)
                    # alpha-only constants into q1, q3 (and stop S)
                    nc.tensor.matmul(
                        S.rearrange("p a b c -> p (a b c)"),
                        lhsT=constM[:, qd, :], rhs=selC,
                        start=False, stop=True, skip_group_check=True)

                    # ---- stats: var -> 1/sqrt ----
                    sqt = sfm.tile([P, 2, 4, NB], f32, name="sqt", tag="sqt")
                    if flags["pool_psum"]:
                        nc.gpsimd.tensor_mul(out=sqt, in0=S[:, 0:2, :, :],
                                             in1=S[:, 0:2, :, :])
                    else:
                        nc.vector.tensor_mul(out=sqt, in0=S[:, 0:2, :, :],
                                             in1=S[:, 0:2, :, :])
                    vvt = sfm.tile([P, 2, 4, NB], f32, name="vvt", tag="vvt")
                    nc.vector.tensor_sub(out=vvt, in0=S[:, 2:4, :, :], in1=sqt)
                    lnv = sfm.tile([P, 2, 4, NB], f32, name="lnv", tag="lnv")
                    nc.scalar.activation(out=lnv, in_=vvt, func=AF.Ln,
                                         bias=epsc, scale=1.0)
                    ivq = sfm.tile([P, 2, 4, NB], f32, name="ivq", tag="ivq")
                    nc.scalar.activation(out=ivq, in_=lnv, func=AF.Exp,
                                         scale=-0.5)

                    # ---- softmax ----
                    stile = sfm.tile([P, 4, K + 1, NB], f32, name="stile", tag="stile")
                    inv_z = ivq[:, 0, :, :].unsqueeze(2).broadcast_to(
                        (P, 4, K, NB))
                    if flags["pool_psum"]:
                        nc.gpsimd.tensor_tensor(out=stile[:, :, 0:K, :],
                                                in0=scp, in1=inv_z, op=OP.mult)
                        nc.gpsimd.tensor_tensor(out=stile[:, :, K, :],
                                                in0=U, in1=ivq[:, 1, :, :],
                                                op=OP.mult)
                    else:
                        nc.vector.tensor_tensor(out=stile[:, :, 0:K, :],
                                                in0=scp, in1=inv_z, op=OP.mult)
                        nc.vector.tensor_tensor(out=stile[:, :, K, :],
                                                in0=U, in1=ivq[:, 1, :, :],
                                                op=OP.mult)
                    etile = sfm.tile([P, 4, K + 1, NB], bf16, name="etile", tag="etile")
                    nc.scalar.activation(out=etile, in_=stile, func=AF.Exp)
                    ev = etile[:, :, 0:K, :]
                    if flags["use_beta0"]:
                        nc.vector.tensor_tensor(
                            out=ev, in0=ev,
                            in1=f3v("ecbrep").unsqueeze(1).unsqueeze(3)
                            .broadcast_to((P, 4, K, NB)), op=OP.mult)
                    sk = sfm.tile([P, 4, NB], f32, name="sk", tag="sk")
                    nc.vector.reduce_sum(
                        out=sk, in_=ev.rearrange("p c k b -> p c b k"),
                        axis=AX.X)
                    rk2 = sfm.tile([P, 4, NB], bf16, name="rk2", tag="rk2")
                    nc.vector.tensor_tensor(out=rk2, in0=etile[:, :, K, :],
                                            in1=sk, op=OP.divide)
                    wt = sfm.tile([P, 4, K, NB], bf16, name="wt", tag="wt")
                    nc.vector.tensor_tensor(
                        out=wt, in0=ev,
                        in1=rk2.unsqueeze(2).broadcast_to((P, 4, K, NB)),
                        op=OP.mult)

                    # ---- aggregation + u-sum ----
                    for c01 in range(4):
                        i = 4 * qd + c01
                        for b in range(NB):
                            nc.tensor.matmul(
                                aggp[:, b, :], lhsT=zg[:, b, i, :],
                                rhs=wt[:, c01, :, b],
                                start=(i == 0), stop=(i == NI - 1),
                                skip_group_check=True)
                        nc.tensor.matmul(
                            spp, lhsT=etile[:, c01, K, :], rhs=ones_bf,
                            start=(i == 0), stop=(i == NI - 1),
                            skip_group_check=True)

                # ---- group tail ----
                srec = gsb.tile([NB, 1], f32, name="srec", tag="srec")
                nc.vector.reciprocal(out=srec, in_=spp)
                srp = psSm.tile([P, 1], f32, tag="sm", name="srp")
                nc.tensor.matmul(srp, lhsT=rep16, rhs=srec, start=True,
                                 stop=True)
                srr = gsb.tile([P, 1], f32, name="srr", tag="srr")
                if flags["pool_psum"]:
                    nc.gpsimd.tensor_copy(out=srr, in_=srp)
                else:
                    nc.scalar.copy(out=srr, in_=srp)
                ats = gsb.tile([D, NB * K], f32, name="ats", tag="ats")
                nc.scalar.copy(out=ats, in_=aggc[0:D, 0:NB * K])
                atp2 = psSm.tile([P, D], f32, tag="sm", name="atp2")
                nc.tensor.transpose(atp2, ats, ident)
                a8 = gsb.tile([P, D], f32, name="a8", tag="a8")
                nc.vector.scalar_tensor_tensor(
                    out=a8, in0=atp2, scalar=srr, in1=bsqrep,
                    op0=OP.mult, op1=OP.add)
                fst = gsb.tile([P, 6], f32, name="fst", tag="fst")
                nc.vector.bn_stats(out=fst, in_=a8)
                fmv = gsb.tile([P, 2], f32, name="fmv", tag="fmv")
                nc.vector.bn_aggr(out=fmv, in_=fst)
                flv = gsb.tile([P, 1], f32, name="flv", tag="flv")
                nc.scalar.activation(out=flv, in_=fmv[:, 1:2], func=AF.Ln,
                                     bias=epsc, scale=1.0)
                fiv = gsb.tile([P, 1], f32, name="fiv", tag="fiv")
                nc.scalar.activation(out=fiv, in_=flv, func=AF.Exp, scale=-0.5)
                obuf = gsb.tile([P, D], f32, name="obuf", tag="obuf")
                nc.vector.tensor_scalar(out=obuf, in0=a8, scalar1=fmv[:, 0:1],
                                        scalar2=fiv, op0=OP.subtract,
                                        op1=OP.mult)
                if flags["use_g4b4"]:
                    nc.vector.tensor_mul(out=obuf, in0=obuf, in1=f3v("g4rep"))
                    nc.vector.tensor_add(out=obuf, in0=obuf, in1=f3v("b4rep"))
                nc.sync.dma_start(
                    out=out_d[g * NB:(g + 1) * NB].flatten_outer_dims(),
                    in_=obuf)

    return nc


def _build(flags):
    import concourse.bacc as bacc
    from concourse import mybir

    _setup_act_tables()
    f32 = mybir.dt.float32
    bf16 = mybir.dt.bfloat16
    bfc, f3c = _param_layouts(flags)
    nc = bacc.Bacc("TRN2", target_bir_lowering=False, debug=False,
                   num_devices=NCORES)
    dp = nc.declare_dram_parameter
    zg_d = dp("zg", [NG, P, NB * NI * D], bf16, isOutput=False)
    zT_d = dp("zT", [NG, P, 4 * NI * P], bf16, isOutput=False)
    pbf_d = dp("pbf", [P, bfc["_total"]], bf16, isOutput=False)
    pf3_d = dp("pf3", [P, f3c["_total"]], f32, isOutput=False)
    out_d = dp("out", [B_CORE, K, D], f32, isOutput=True)
    _emit(nc, zg_d, zT_d, pbf_d, pf3_d, out_d, flags, bfc, f3c)
    nc.finalize()
    return nc


def _param_layouts(flags):
    bfc = {}
    o = 0
    for name, cols in [("aT2", NI * P), ("rq", 4), ("raz", 2),
                       ("RG", 2 * K), ("constM", 2 * P), ("selC", P), ("idbf", D),
                       ("WIC", D), ("ones", 1)]:
        bfc[name] = (o, cols)
        o += cols
    bfc["_total"] = o
    f3c = {}
    o = 0
    names = [("zlast", D), ("ab8rep", D), ("g2col", 1), ("ident", D),
             ("rep16", P), ("bsqrep", D)]
    if flags["use_beta0"]:
        names.append(("ecbrep", K))
    if flags["use_g3b3"]:
        names += [("g3rep", D), ("b3rep", D)]
    if flags["use_g4b4"]:
        names += [("g4rep", D), ("b4rep", D)]
    for name, cols in names:
        f3c[name] = (o, cols)
        o += cols
    f3c["_total"] = o
    return bfc, f3c


def _ln_np(x, g, b):
    m = x.mean(axis=-1, keepdims=True)
    v = ((x - m) ** 2).mean(axis=-1, keepdims=True)
    return (x - m) / np.sqrt(v + EPS) * g + b


def _host_prep(inputs, flags, bfc, f3c):
    """Shared (non-z) parameter buffers."""
    import ml_dtypes
    bf = ml_dtypes.bfloat16

    al = np.asarray(inputs["alphas"], np.float32)        # [T, D]
    proto = np.asarray(inputs["prototypes"], np.float32)
    bbias = np.asarray(inputs["b_bias"], np.float32)
    W = np.asarray(inputs["W"], np.float32)
    gam = np.asarray(inputs["ln_gamma"], np.float32)
    bet = np.asarray(inputs["ln_beta"], np.float32)
    bseq = np.asarray(inputs["beta_seq"], np.float32)

    pn = _ln_np(proto, gam[1], bet[1])                   # [K, D]
    G = (pn * gam[0]).T / 8.0                            # [D, K]
    Gc = G - G.mean(axis=0, keepdims=True)               # center: kills m*cg

    alp = al.reshape(P, NI, D)                           # [tau, i, d]

    pbf = np.zeros((P, bfc["_total"]), np.float32)

    def put(name, rows, arr):
        off, ncol = bfc[name]
        pbf[0:rows, off:off + ncol] = arr.reshape(rows, ncol)

    aT2h = alp.transpose(2, 1, 0)                        # [d, i, tau]
    aT2h = np.concatenate([aT2h, aT2h], axis=0)          # [128, 8, 128]
    put("aT2", P, aT2h)
    rqh = np.zeros((P, 2, 2), np.float32)                # [p, q01, b2]
    for b2 in range(2):
        rqh[b2 * D:(b2 + 1) * D, :, b2] = 1.0 / 64.0
    put("rq", P, rqh)
    razh = np.zeros((P, 2), np.float32)
    for b2 in range(2):
        razh[b2 * D:(b2 + 1) * D, b2] = 2.0 / 64.0
    put("raz", P, razh)
    RGh = np.zeros((P, K, 2), np.float32)                # [p, k, b2]
    for b2 in range(2):
        RGh[b2 * D:(b2 + 1) * D, :, b2] = Gc
    put("RG", P, RGh)
    ras64 = alp.sum(axis=2).T / 64.0                     # [i, tau]
    ras264 = (alp ** 2).sum(axis=2).T / 64.0             # [i, tau]
    constMh = np.zeros((8, 2, P), np.float32)            # [2ii+v, qd, tau]
    for qd in range(2):
        for ii in range(4):
            constMh[2 * ii + 0, qd] = ras64[4 * qd + ii]
            constMh[2 * ii + 1, qd] = ras264[4 * qd + ii]
    put("constM", 8, constMh)
    put("idbf", D, np.eye(D, dtype=np.float32))
    selCh = np.zeros((8, 4, 4, NB), np.float32)          # [row, q, ch, b]
    for ii in range(4):
        selCh[2 * ii + 0, 1, ii, :] = 1.0                # ras64 -> q1 (mza)
        selCh[2 * ii + 1, 3, ii, :] = 1.0                # ras264 -> q3
    put("selC", 8, selCh)
    C = np.eye(D, dtype=np.float32) - 1.0 / 64.0         # I - J/64
    WICh = C @ (np.eye(D, dtype=np.float32) + W)         # hc = C(I+W)q
    put("WIC", D, WICh)
    put("ones", P, np.ones((P, 1), np.float32))
    pbf = pbf.astype(bf)

    pf3 = np.zeros((P, f3c["_total"]), np.float32)

    def putf(name, rows, arr):
        off, ncol = f3c[name]
        pf3[0:rows, off:off + ncol] = arr.reshape(rows, ncol)

    putf("ab8rep", D, np.broadcast_to(al[-1] + bbias, (D, D)).copy())
    putf("g2col", D, (gam[2] / 8.0).reshape(D, 1))
    putf("ident", D, np.eye(D, dtype=np.float32))
    rep16h = np.zeros((NB, P), np.float32)
    for b in range(NB):
        rep16h[b, b * K:(b + 1) * K] = 1.0
    putf("rep16", NB, rep16h)
    putf("bsqrep", P, np.broadcast_to(
        bseq[None, :, :], (NB, K, D)).reshape(P, D).copy())
    if flags["use_beta0"]:
        cb = pn @ bet[0]                                 # [K]
        putf("ecbrep", P, np.broadcast_to(np.exp(cb / 1.0)[None, :],
                                          (P, K)).copy())
    if flags["use_g3b3"]:
        putf("g3rep", D, np.broadcast_to(gam[3], (D, D)).copy())
        putf("b3rep", D, np.broadcast_to(bet[3], (D, D)).copy())
    if flags["use_g4b4"]:
        putf("g4rep", P, np.broadcast_to(gam[4], (P, D)).copy())
        putf("b4rep", P, np.broadcast_to(bet[4], (P, D)).copy())
    return pbf, pf3


def kernel(**inputs):
    import ml_dtypes
    from concourse.bass_utils import run_bass_kernel_spmd

    bf = ml_dtypes.bfloat16
    z = np.ascontiguousarray(inputs["z"], dtype=np.float32)
    gam = np.asarray(inputs["ln_gamma"], np.float32)
    bet = np.asarray(inputs["ln_beta"], np.float32)
    flags = {
        "use_beta0": bool(np.abs(bet[0]).max() > 0),
        "use_g3b3": bool(np.abs(gam[3] - 1).max() > 0
                         or np.abs(bet[3]).max() > 0),
        "use_g4b4": bool(np.abs(gam[4] - 1).max() > 0
                         or np.abs(bet[4]).max() > 0),
        "pool_psum": True,
    }
    key = tuple(sorted(flags.items()))
    if key not in _CACHE:
        _CACHE[key] = _build(flags)
    nc = _CACHE[key]

    bfc, f3c = _param_layouts(flags)
    pbf, pf3_base = _host_prep(inputs, flags, bfc, f3c)

    in_maps = []
    for c in range(NCORES):
        zc = z[c * B_CORE:(c + 1) * B_CORE]              # [64, 1024, 64]
        zc5 = zc.reshape(NG, NB, P, NI, D)
        zg_nat = np.ascontiguousarray(
            zc5.transpose(0, 2, 1, 3, 4)).reshape(NG, P, NB * NI * D)
        zc6 = zc.reshape(NG, 4, 2, P, NI, D)             # [g, j, b2, tau, i, d]
        zT = np.ascontiguousarray(
            zc6.transpose(0, 2, 5, 1, 4, 3)).reshape(NG, P, 4 * NI * P)
        pf3 = pf3_base.copy()
        off, ncol = f3c["zlast"]
        pf3[0:D, off:off + ncol] = zc[:, -1, :]
        in_maps.append({
            "zg": zg_nat.astype(bf),
            "zT": zT.astype(bf),
            "pbf": pbf,
            "pf3": pf3,
        })
    res = run_bass_kernel_spmd(nc, in_maps, core_ids=list(range(NCORES)))
    out = np.concatenate([r["out"] for r in res.results], axis=0)
    return out


# revision 9
# speedup vs baseline: 3.1362x; 1.0627x over previous
"""DisentangledSeqEncoder Trainium2 kernel, v2.

Strategy (pure data-parallel over batch, 8 NeuronCores):
  - Host sends z in TWO bf16 layouts (natural + transposed), both with
    >= 8KB-contiguous rows so every big DMA runs at full bus rate.
    Token t = 8*tau + i lives at partition tau, chunk i in 0..7.
  - All per-(token,batch) reductions are PE matmul columns against the
    transposed z:
      * scores_k  = z @ Gc          (Gc column-centered: kills the m*cg term)
      * u-col     = (z+a) @ hc      (hc = C(I+W)q with C = I - J/64: the
                                     centering kills the m*sh term exactly)
      * moments   = [Sz/64, S(z+a)/64, Sz^2/64, S((z+a)^2)/64] via matmuls
        on z, z*z (DVE/ACT elementwise), and a*z (DVE), plus a constant
        matmul adding alpha-only terms.
  - Softmax: var -> Ln/Exp rsqrt on ACT; scale on Pool (PSUM-read);
    exp on ACT; k-sum on Pool; u/sk and ev*(u/sk) on DVE (bf16 2x mode).
  - Aggregation: per (chunk,batch) matmul lhsT=z_nat rhs=wt -> [d,(b,k)]
    PSUM accumulated over all 1024 tokens; group tail does the final LN.
  - The whole q->h chain runs ONCE for all 8 groups at startup.
  - gamma/beta LN params are folded on the host into Gc/g2col (exact);
    runtime flags add extra ops only when beta0 / gamma3,beta3 /
    gamma4,beta4 are nontrivial (they are ones/zeros here).
"""

import numpy as np

EPS = 1e-6
B_FULL, T, D, K = 512, 1024, 64, 16
NCORES = 8
B_CORE = B_FULL // NCORES          # 64
NG = 8                             # batch groups per core
NB = 8                             # batches per group
NI = 8                             # chunks (inner token index)
P = 128                            # partitions

_CACHE = {}


def _setup_act_tables():
    """Reorder act_func_sets so natural_log_exp_and_others is first (avoids
    per-chunk ACT_TABLE_LOAD thrash on real hw)."""
    import os
    import json
    import functools
    import concourse.hw_specs as hw_specs
    import concourse.bacc as bacc

    if getattr(_setup_act_tables, "_done", False):
        return
    from neuronxcc.driver.Job import Job
    from neuronxcc.driver.jobs.support.FindActInfo import findActInfoFile

    src = findActInfoFile(Job.getPackageDir(), "gen3")
    srcdir = os.path.dirname(src)
    info = json.load(open(src))
    sets = info["act_func_sets"]
    sets.sort(key=lambda e: 0 if e["name"] == "natural_log_exp_and_others" else 1)
    dst = "/tmp/act_reordered"
    os.makedirs(dst, exist_ok=True)
    tmp = os.path.join(dst, f"act_info.{os.getpid()}.tmp")
    json.dump(info, open(tmp, "w"))
    os.replace(tmp, os.path.join(dst, "act_info.json"))
    for f in os.listdir(srcdir):
        if f.endswith(".bin") or f.endswith(".json"):
            l = os.path.join(dst, f)
            if f != "act_info.json" and not os.path.exists(l):
                try:
                    os.symlink(os.path.join(srcdir, f), l)
                except FileExistsError:
                    pass
    os.environ["BASS_ACT_ROOT_JSON_PATH"] = os.path.join(dst, "act_info.json")

    orig = hw_specs.get_activation_tables

    @functools.cache
    def patched(arch):
        d = dict(orig(arch))
        items = list(d.items())
        items.sort(key=lambda kv: 0 if kv[0] == "natural_log_exp_and_others"
                   else 1)
        return dict(items)

    hw_specs.get_activation_tables = patched
    bacc.get_activation_tables = patched
    _setup_act_tables._done = True


def _emit(nc, zg_d, zT_d, pbf_d, pf3_d, out_d, flags, bfc, f3c):
    import concourse.tile as tile
    import concourse.bass as bass
    from concourse import mybir

    f32 = mybir.dt.float32
    bf16 = mybir.dt.bfloat16
    OP = mybir.AluOpType
    AF = mybir.ActivationFunctionType
    AX = mybir.AxisListType

    NBF = bfc["_total"]
    NF3 = f3c["_total"]

    with tile.TileContext(nc) as tc:
        with (
            tc.tile_pool(name="singles", bufs=1) as singles,
            tc.tile_pool(name="zn", bufs=4) as znp,
            tc.tile_pool(name="zt", bufs=4) as ztp_pool,
            tc.tile_pool(name="prod", bufs=3) as prod,
            tc.tile_pool(name="sfm", bufs=3) as sfm,
            tc.tile_pool(name="gsb", bufs=2) as gsb,
            tc.tile_pool(name="psS", bufs=3, space="PSUM") as psS,
            tc.tile_pool(name="psC", bufs=3, space="PSUM") as psC,   # scores
            tc.tile_pool(name="psAgg", bufs=2, space="PSUM") as psAgg,
        ):
            # ================= startup =================
            pbf = singles.tile([P, NBF], bf16)
            nc.sync.dma_start(out=pbf, in_=pbf_d[:, :])
            pf3 = singles.tile([P, NF3], f32)
            nc.sync.dma_start(out=pf3, in_=pf3_d[:, :])

            def bfv(name, rows=P):
                off, ncol = bfc[name]
                return pbf[0:rows, off:off + ncol]

            def f3v(name, rows=P):
                off, ncol = f3c[name]
                return pf3[0:rows, off:off + ncol]

            aT2 = bfv("aT2").rearrange("p (i t) -> p i t", i=NI)
            rq = bfv("rq")
            raz = bfv("raz")
            RG = bfv("RG")
            constM = bfv("constM", rows=8).rearrange("p (q t) -> p q t", q=2)
            selC = bfv("selC", rows=8)
            WIC = bfv("WIC", rows=D)
            idbf = bfv("idbf", rows=D)
            ones_bf = bfv("ones")

            zlast = f3v("zlast", rows=D)
            ab8rep = f3v("ab8rep", rows=D)
            g2col = f3v("g2col", rows=D)
            ident = f3v("ident", rows=D)
            rep16 = f3v("rep16", rows=NB)
            bsqrep = f3v("bsqrep")

            epsc = singles.tile([P, 1], f32)
            nc.vector.memset(epsc, EPS)

            # ---- q -> hc chain, once for all 64 (g,b) ----
            qin = singles.tile([D, D], f32)
            nc.vector.tensor_add(out=qin, in0=zlast, in1=ab8rep)
            qst = singles.tile([D, 6], f32)
            nc.vector.bn_stats(out=qst, in_=qin)
            qmv = singles.tile([D, 2], f32)
            nc.vector.bn_aggr(out=qmv, in_=qst)
            qlv = singles.tile([D, 1], f32)
            nc.scalar.activation(out=qlv, in_=qmv[:, 1:2], func=AF.Ln,
                                 bias=epsc[0:D], scale=1.0)
            qiv = singles.tile([D, 1], f32)
            nc.scalar.activation(out=qiv, in_=qlv, func=AF.Exp, scale=-0.5)
            q_t = singles.tile([D, D], f32)
            nc.vector.tensor_scalar(out=q_t, in0=qin, scalar1=qmv[:, 0:1],
                                    scalar2=qiv, op0=OP.subtract, op1=OP.mult)
            if flags["use_g3b3"]:
                nc.vector.tensor_mul(out=q_t, in0=q_t, in1=f3v("g3rep", rows=D))
                nc.vector.tensor_add(out=q_t, in0=q_t, in1=f3v("b3rep", rows=D))
            qtp = psS.tile([D, D], f32, tag="S2q", name="qtp")
            nc.tensor.transpose(qtp, q_t, ident)
            qts = singles.tile([D, D], bf16)
            nc.scalar.copy(out=qts, in_=qtp)
            h1p = psS.tile([D, D], f32, tag="S2q", name="h1p")
            nc.tensor.matmul(h1p, lhsT=WIC, rhs=qts, start=True, stop=True)
            hT8 = singles.tile([D, D], bf16)
            nc.vector.tensor_scalar_mul(out=hT8, in0=h1p, scalar1=g2col)
            # block-diag h columns for the u-col matmuls: [(b2,d), g, j, b2']
            hcp = psS.tile([P, NG, 4, 2], f32, tag="S2q", name="hcp")
            nc.tensor.matmul(
                hcp[0:D, :, :, 0].rearrange("p a b -> p (a b)"), lhsT=idbf,
                rhs=hT8[:, 0::2], start=True, stop=True,
                skip_group_check=True)
            nc.tensor.matmul(
                hcp[D:P, :, :, 1].rearrange("p a b -> p (a b)"),
                lhsT=idbf, rhs=hT8[:, 1::2], start=True, stop=True,
                skip_group_check=True)
            nc.vector.memset(hcp[0:D, :, :, 1], 0.0)
            nc.vector.memset(hcp[D:P, :, :, 0], 0.0)
            hcall = singles.tile([P, NG, 4, 2], bf16)
            nc.scalar.copy(out=hcall, in_=hcp)

            # ================= group loop =================
            for g in range(NG):
                zTf = ztp_pool.tile([P, NI * 4 * P], bf16, name="zTf")
                nc.sync.dma_start(out=zTf, in_=zT_d[g, :, :])
                zT = zTf.rearrange("p (j i t) -> p j i t", j=4, i=NI)
                zgf = znp.tile([P, NB * NI * D], bf16, name="zgf")
                nc.sync.dma_start(out=zgf, in_=zg_d[g, :, :])
                zg = zgf.rearrange("p (b i d) -> p b i d", b=NB, i=NI)

                hT8g = hT8[:, g * NB:(g + 1) * NB]
                aggc = psAgg.tile([P, NB * K + D + 2], f32, name="aggc")
                aggp = aggc[0:D, 0:NB * K].rearrange("p (b k) -> p b k", b=NB)
                spp = aggc[0:NB, NB * K:NB * K + 1]
                atp2 = aggc[:, NB * K + 1:NB * K + 1 + D]
                srp = aggc[:, NB * K + 1 + D:NB * K + 2 + D]

                for qd in range(2):
                    S2 = psS.tile([P, 5, 4, NB], f32, tag="S2q", name="S2")
                    S = S2[:, 0:4, :, :]
                    U = S2[:, 4, :, :]
                    scp = psC.tile([P, 4, K, NB], f32, name="scp")

                    zzp = prod.tile([P, 4, 2, 2, P], bf16, name="zzp", tag="zzp")
                    azp = prod.tile([P, 4, 2, 2, P], bf16, name="azp", tag="azp")
                    for cc in range(2):
                        i0 = 4 * qd + 2 * cc
                        zsl = zT[:, :, i0:i0 + 2, :]
                        asl = aT2[:, i0:i0 + 2, :].unsqueeze(1) \
                            .broadcast_to((P, 4, 2, P))
                        # z*z : half on ACT (Square), half on Pool
                        nc.scalar.activation(out=zzp[:, 0:2, cc],
                                             in_=zT[:, 0:2, i0:i0 + 2, :],
                                             func=AF.Square)
                        nc.gpsimd.tensor_mul(out=zzp[:, 2:4, cc],
                                             in0=zT[:, 2:4, i0:i0 + 2, :],
                                             in1=zT[:, 2:4, i0:i0 + 2, :])
                        # a*z on DVE (bf16 2x)
                        nc.vector.tensor_tensor(out=azp[:, :, cc], in0=zsl,
                                                in1=asl, op=OP.mult)

                        for c01 in range(2):
                            i = i0 + c01
                            ch = 2 * cc + c01
                            for j in range(4):
                                # moments: q0=Sz/64, q1=S(z+a)/64 (partial),
                                # q2=Sz2/64, q3=S((z+a)^2)/64 (partial)
                                nc.tensor.matmul(
                                    S[:, 0:2, ch, 2 * j:2 * j + 2],
                                    lhsT=zT[:, j, i, :], rhs=rq,
                                    start=True, stop=False,
                                    skip_group_check=True)
                                nc.tensor.matmul(
                                    S[:, 2:4, ch, 2 * j:2 * j + 2],
                                    lhsT=zzp[:, j, cc, c01, :], rhs=rq,
                                    start=True, stop=False,
                                    skip_group_check=True)
                                nc.tensor.matmul(
                                    S[:, 3, ch, 2 * j:2 * j + 2],
                                    lhsT=azp[:, j, cc, c01, :], rhs=raz,
                                    start=False, stop=False,
                                    skip_group_check=True)
                                # scores: z @ Gc
                                nc.tensor.matmul(
                                    scp[:, ch, :, 2 * j:2 * j + 2],
                                    lhsT=zT[:, j, i, :], rhs=RG,
                                    start=True, stop=True,
                                    skip_group_check=True)
                                # u-col: z @ hc (block-diag h cols)
                                nc.tensor.matmul(
                                    U[:, ch, 2 * j:2 * j + 2],
                                    lhsT=zT[:, j, i, :],
                                    rhs=hcall[:, g, j, :],
                                    start=True, stop=False,
                                    skip_group_check=True)
                            # u-col: + a @ hc
                            nc.tensor.matmul(
                                U[:, ch, :], lhsT=aT2[0:D, i, :], rhs=hT8g,
                                start=False, stop=True, skip_group_check=True)
                    # alpha-only constants into q1, q3 (and stop S)
                    nc.tensor.matmul(
                        S.rearrange("p a b c -> p (a b c)"),
                        lhsT=constM[:, qd, :], rhs=selC,
                        start=False, stop=True, skip_group_check=True)

                    # ---- stats: var -> 1/sqrt ----
                    sqt = sfm.tile([P, 2, 4, NB], f32, name="sqt", tag="sqt")
                    nc.vector.tensor_mul(out=sqt, in0=S[:, 0:2, :, :],
                                         in1=S[:, 0:2, :, :])
                    vvt = sfm.tile([P, 2, 4, NB], f32, name="vvt", tag="vvt")
                    nc.vector.tensor_sub(out=vvt, in0=S[:, 2:4, :, :], in1=sqt)
                    lnv = sfm.tile([P, 2, 4, NB], f32, name="lnv", tag="lnv")
                    nc.scalar.activation(out=lnv, in_=vvt, func=AF.Ln,
                                         bias=epsc, scale=1.0)
                    ivq = sfm.tile([P, 2, 4, NB], f32, name="ivq", tag="ivq")
                    nc.scalar.activation(out=ivq, in_=lnv, func=AF.Exp,
                                         scale=-0.5)

                    # ---- softmax ----
                    stile = sfm.tile([P, 4, K + 1, NB], f32, name="stile", tag="stile")
                    inv_z = ivq[:, 0, :, :].unsqueeze(2).broadcast_to(
                        (P, 4, K, NB))
                    if qd % 2 == 0:
                        nc.vector.tensor_tensor(out=stile[:, :, 0:K, :],
                                                in0=scp, in1=inv_z, op=OP.mult)
                    else:
                        scpc = sfm.tile([P, 4, K, NB], f32, name="scpc",
                                        tag="scpc")
                        nc.scalar.copy(out=scpc, in_=scp)
                        nc.gpsimd.tensor_tensor(out=stile[:, :, 0:K, :],
                                                in0=scpc, in1=inv_z,
                                                op=OP.mult)
                    nc.vector.tensor_tensor(out=stile[:, :, K, :],
                                            in0=U, in1=ivq[:, 1, :, :],
                                            op=OP.mult)
                    etile = sfm.tile([P, 4, K + 1, NB], bf16, name="etile", tag="etile")
                    nc.scalar.activation(out=etile, in_=stile, func=AF.Exp)
                    ev = etile[:, :, 0:K, :]
                    if flags["use_beta0"]:
                        nc.vector.tensor_tensor(
                            out=ev, in0=ev,
                            in1=f3v("ecbrep").unsqueeze(1).unsqueeze(3)
                            .broadcast_to((P, 4, K, NB)), op=OP.mult)
                    sk = sfm.tile([P, 4, NB], f32, name="sk", tag="sk")
                    nc.vector.reduce_sum(
                        out=sk, in_=ev.rearrange("p c k b -> p c b k"),
                        axis=AX.X)
                    rk2 = sfm.tile([P, 4, NB], bf16, name="rk2", tag="rk2")
                    nc.vector.tensor_tensor(out=rk2, in0=etile[:, :, K, :],
                                            in1=sk, op=OP.divide)
                    wt = sfm.tile([P, 4, K, NB], bf16, name="wt", tag="wt")
                    nc.vector.tensor_tensor(
                        out=wt, in0=ev,
                        in1=rk2.unsqueeze(2).broadcast_to((P, 4, K, NB)),
                        op=OP.mult)

                    # ---- aggregation + u-sum ----
                    for c01 in range(4):
                        i = 4 * qd + c01
                        for b in range(NB):
                            nc.tensor.matmul(
                                aggp[:, b, :], lhsT=zg[:, b, i, :],
                                rhs=wt[:, c01, :, b],
                                start=(i == 0), stop=(i == NI - 1),
                                skip_group_check=True)
                        nc.tensor.matmul(
                            spp, lhsT=etile[:, c01, K, :], rhs=ones_bf,
                            start=(i == 0), stop=(i == NI - 1),
                            skip_group_check=True)

                # ---- group tail ----
                srec = gsb.tile([NB, 1], f32, name="srec", tag="srec")
                nc.vector.reciprocal(out=srec, in_=spp)
                nc.tensor.matmul(srp, lhsT=rep16, rhs=srec, start=True,
                                 stop=True)
                srr = gsb.tile([P, 1], f32, name="srr", tag="srr")
                nc.scalar.copy(out=srr, in_=srp)
                ats = gsb.tile([D, NB * K], f32, name="ats", tag="ats")
                nc.scalar.copy(out=ats, in_=aggc[0:D, 0:NB * K])
                nc.tensor.transpose(atp2, ats, ident)
                a8 = gsb.tile([P, D], f32, name="a8", tag="a8")
                nc.vector.scalar_tensor_tensor(
                    out=a8, in0=atp2, scalar=srr, in1=bsqrep,
                    op0=OP.mult, op1=OP.add)
                fst = gsb.tile([P, 6], f32, name="fst", tag="fst")
                nc.vector.bn_stats(out=fst, in_=a8)
                fmv = gsb.tile([P, 2], f32, name="fmv", tag="fmv")
                nc.vector.bn_aggr(out=fmv, in_=fst)
                flv = gsb.tile([P, 1], f32, name="flv", tag="flv")
                nc.scalar.activation(out=flv, in_=fmv[:, 1:2], func=AF.Ln,
                                     bias=epsc, scale=1.0)
                fiv = gsb.tile([P, 1], f32, name="fiv", tag="fiv")
                nc.scalar.activation(out=fiv, in_=flv, func=AF.Exp, scale=-0.5)
                obuf = gsb.tile([P, D], f32, name="obuf", tag="obuf")
                nc.vector.tensor_scalar(out=obuf, in0=a8, scalar1=fmv[:, 0:1],
                                        scalar2=fiv, op0=OP.subtract,
                                        op1=OP.mult)
                if flags["use_g4b4"]:
                    nc.vector.tensor_mul(out=obuf, in0=obuf, in1=f3v("g4rep"))
                    nc.vector.tensor_add(out=obuf, in0=obuf, in1=f3v("b4rep"))
                nc.sync.dma_start(
                    out=out_d[g * NB:(g + 1) * NB].flatten_outer_dims(),
                    in_=obuf)

    return nc


def _build(flags):
    import concourse.bacc as bacc
    from concourse import mybir

    _setup_act_tables()
    f32 = mybir.dt.float32
    bf16 = mybir.dt.bfloat16
    bfc, f3c = _param_layouts(flags)
    nc = bacc.Bacc("TRN2", target_bir_lowering=False, debug=False,
                   num_devices=NCORES)
    dp = nc.declare_dram_parameter
    zg_d = dp("zg", [NG, P, NB * NI * D], bf16, isOutput=False)
    zT_d = dp("zT", [NG, P, 4 * NI * P], bf16, isOutput=False)
    pbf_d = dp("pbf", [P, bfc["_total"]], bf16, isOutput=False)
    pf3_d = dp("pf3", [P, f3c["_total"]], f32, isOutput=False)
    out_d = dp("out", [B_CORE, K, D], f32, isOutput=True)
    _emit(nc, zg_d, zT_d, pbf_d, pf3_d, out_d, flags, bfc, f3c)
    nc.finalize()
    return nc


def _param_layouts(flags):
    bfc = {}
    o = 0
    for name, cols in [("aT2", NI * P), ("rq", 4), ("raz", 2),
                       ("RG", 2 * K), ("constM", 2 * P), ("selC", P), ("idbf", D),
                       ("WIC", D), ("ones", 1)]:
        bfc[name] = (o, cols)
        o += cols
    bfc["_total"] = o
    f3c = {}
    o = 0
    names = [("zlast", D), ("ab8rep", D), ("g2col", 1), ("ident", D),
             ("rep16", P), ("bsqrep", D)]
    if flags["use_beta0"]:
        names.append(("ecbrep", K))
    if flags["use_g3b3"]:
        names += [("g3rep", D), ("b3rep", D)]
    if flags["use_g4b4"]:
        names += [("g4rep", D), ("b4rep", D)]
    for name, cols in names:
        f3c[name] = (o, cols)
        o += cols
    f3c["_total"] = o
    return bfc, f3c


def _ln_np(x, g, b):
    m = x.mean(axis=-1, keepdims=True)
    v = ((x - m) ** 2).mean(axis=-1, keepdims=True)
    return (x - m) / np.sqrt(v + EPS) * g + b


def _host_prep(inputs, flags, bfc, f3c):
    """Shared (non-z) parameter buffers."""
    import ml_dtypes
    bf = ml_dtypes.bfloat16

    al = np.asarray(inputs["alphas"], np.float32)        # [T, D]
    proto = np.asarray(inputs["prototypes"], np.float32)
    bbias = np.asarray(inputs["b_bias"], np.float32)
    W = np.asarray(inputs["W"], np.float32)
    gam = np.asarray(inputs["ln_gamma"], np.float32)
    bet = np.asarray(inputs["ln_beta"], np.float32)
    bseq = np.asarray(inputs["beta_seq"], np.float32)

    pn = _ln_np(proto, gam[1], bet[1])                   # [K, D]
    G = (pn * gam[0]).T / 8.0                            # [D, K]
    Gc = G - G.mean(axis=0, keepdims=True)               # center: kills m*cg

    alp = al.reshape(P, NI, D)                           # [tau, i, d]

    pbf = np.zeros((P, bfc["_total"]), np.float32)

    def put(name, rows, arr):
        off, ncol = bfc[name]
        pbf[0:rows, off:off + ncol] = arr.reshape(rows, ncol)

    aT2h = alp.transpose(2, 1, 0)                        # [d, i, tau]
    aT2h = np.concatenate([aT2h, aT2h], axis=0)          # [128, 8, 128]
    put("aT2", P, aT2h)
    rqh = np.zeros((P, 2, 2), np.float32)                # [p, q01, b2]
    for b2 in range(2):
        rqh[b2 * D:(b2 + 1) * D, :, b2] = 1.0 / 64.0
    put("rq", P, rqh)
    razh = np.zeros((P, 2), np.float32)
    for b2 in range(2):
        razh[b2 * D:(b2 + 1) * D, b2] = 2.0 / 64.0
    put("raz", P, razh)
    RGh = np.zeros((P, K, 2), np.float32)                # [p, k, b2]
    for b2 in range(2):
        RGh[b2 * D:(b2 + 1) * D, :, b2] = Gc
    put("RG", P, RGh)
    ras64 = alp.sum(axis=2).T / 64.0                     # [i, tau]
    ras264 = (alp ** 2).sum(axis=2).T / 64.0             # [i, tau]
    constMh = np.zeros((8, 2, P), np.float32)            # [2ii+v, qd, tau]
    for qd in range(2):
        for ii in range(4):
            constMh[2 * ii + 0, qd] = ras64[4 * qd + ii]
            constMh[2 * ii + 1, qd] = ras264[4 * qd + ii]
    put("constM", 8, constMh)
    put("idbf", D, np.eye(D, dtype=np.float32))
    selCh = np.zeros((8, 4, 4, NB), np.float32)          # [row, q, ch, b]
    for ii in range(4):
        selCh[2 * ii + 0, 1, ii, :] = 1.0                # ras64 -> q1 (mza)
        selCh[2 * ii + 1, 3, ii, :] = 1.0                # ras264 -> q3
    put("selC", 8, selCh)
    C = np.eye(D, dtype=np.float32) - 1.0 / 64.0         # I - J/64
    WICh = C @ (np.eye(D, dtype=np.float32) + W)         # hc = C(I+W)q
    put("WIC", D, WICh)
    put("ones", P, np.ones((P, 1), np.float32))
    pbf = pbf.astype(bf)

    pf3 = np.zeros((P, f3c["_total"]), np.float32)

    def putf(name, rows, arr):
        off, ncol = f3c[name]
        pf3[0:rows, off:off + ncol] = arr.reshape(rows, ncol)

    putf("ab8rep", D, np.broadcast_to(al[-1] + bbias, (D, D)).copy())
    putf("g2col", D, (gam[2] / 8.0).reshape(D, 1))
    putf("ident", D, np.eye(D, dtype=np.float32))
    rep16h = np.zeros((NB, P), np.float32)
    for b in range(NB):
        rep16h[b, b * K:(b + 1) * K] = 1.0
    putf("rep16", NB, rep16h)
    putf("bsqrep", P, np.broadcast_to(
        bseq[None, :, :], (NB, K, D)).reshape(P, D).copy())
    if flags["use_beta0"]:
        cb = pn @ bet[0]                                 # [K]
        putf("ecbrep", P, np.broadcast_to(np.exp(cb / 1.0)[None, :],
                                          (P, K)).copy())
    if flags["use_g3b3"]:
        putf("g3rep", D, np.broadcast_to(gam[3], (D, D)).copy())
        putf("b3rep", D, np.broadcast_to(bet[3], (D, D)).copy())
    if flags["use_g4b4"]:
        putf("g4rep", P, np.broadcast_to(gam[4], (P, D)).copy())
        putf("b4rep", P, np.broadcast_to(bet[4], (P, D)).copy())
    return pbf, pf3


def kernel(**inputs):
    import ml_dtypes
    from concourse.bass_utils import run_bass_kernel_spmd

    bf = ml_dtypes.bfloat16
    z = np.ascontiguousarray(inputs["z"], dtype=np.float32)
    gam = np.asarray(inputs["ln_gamma"], np.float32)
    bet = np.asarray(inputs["ln_beta"], np.float32)
    flags = {
        "use_beta0": bool(np.abs(bet[0]).max() > 0),
        "use_g3b3": bool(np.abs(gam[3] - 1).max() > 0
                         or np.abs(bet[3]).max() > 0),
        "use_g4b4": bool(np.abs(gam[4] - 1).max() > 0
                         or np.abs(bet[4]).max() > 0),
        "pool_psum": True,
    }
    key = tuple(sorted(flags.items()))
    if key not in _CACHE:
        _CACHE[key] = _build(flags)
    nc = _CACHE[key]

    bfc, f3c = _param_layouts(flags)
    pbf, pf3_base = _host_prep(inputs, flags, bfc, f3c)

    in_maps = []
    for c in range(NCORES):
        zc = z[c * B_CORE:(c + 1) * B_CORE]              # [64, 1024, 64]
        zc5 = zc.reshape(NG, NB, P, NI, D)
        zg_nat = np.ascontiguousarray(
            zc5.transpose(0, 2, 1, 3, 4)).reshape(NG, P, NB * NI * D)
        zc6 = zc.reshape(NG, 4, 2, P, NI, D)             # [g, j, b2, tau, i, d]
        zT = np.ascontiguousarray(
            zc6.transpose(0, 2, 5, 1, 4, 3)).reshape(NG, P, 4 * NI * P)
        pf3 = pf3_base.copy()
        off, ncol = f3c["zlast"]
        pf3[0:D, off:off + ncol] = zc[:, -1, :]
        in_maps.append({
            "zg": zg_nat.astype(bf),
            "zT": zT.astype(bf),
            "pbf": pbf,
            "pf3": pf3,
        })
    res = run_bass_kernel_spmd(nc, in_maps, core_ids=list(range(NCORES)))
    out = np.concatenate([r["out"] for r in res.results], axis=0)
    return out
